# revision 7
# baseline (speedup 1.0000x reference)
"""Trainium2 kernel for nn_ClauseFunction (segment_reduce):
C[b,g] = softor_s(softand_l(x[b, I_i[g,s,l]])), gamma=1e-3.

Strategy: shard over G (each of 8 cores handles 256 g-columns; x replicated).
Per core: gather 256*32*8 = 65536 rows of xT (one row = x[:,j] for all 64 b,
256 bytes f32) from DRAM via gpsimd.dma_gather (64 calls x 1024 idxs), then
logsumexp reductions on DVE/ACT:
  stage1 (over l=8):  m=min_l g; S=sum_l exp((m-g)*1000); v=1000*m - ln S
  stage2 (over s=32): M=max_s v; C=1e-3*(M + ln sum_s exp(v-M))
Layout: gathered tile [128 part, slots, 64 b]; partition p holds g' in
{2p, 2p+1}; slot group c = gl*32+s (gl=g' parity, s); call c gathers l=0..7
for group c of every partition.
"""

import numpy as np

import concourse.bacc as bacc
import concourse.bass as bass
import concourse.tile as tile
from concourse import mybir
from concourse.bass_utils import run_bass_kernel_spmd

B, G, S, L = 64, 2048, 32, 8
NCORES = 8
GSH = G // NCORES  # 256 g' per core
NIDX = 1024  # indices per dma_gather call (ucode scratch-safe)
NCALL = (GSH * S * L) // NIDX  # 64 calls
CALLS_PER_CHUNK = 4  # chunk = 4 calls; caps in-flight gather descriptors
NCHUNK = NCALL // CALLS_PER_CHUNK  # 8
GRP_PER_PART = GSH // 128 * S  # 64 groups (gl, s) per partition

_nc_cache = None
last_result = None


def _v(t, dims):
    """View of tile t with explicit free-dim [stride, count] pairs (elements).

    Keeps the tile's own partition entry (stride = per-partition size)."""
    return bass.AP(tensor=t.tensor, offset=t.offset, ap=[list(t.ap[0])] + dims)


def _build_nc():
    f32 = mybir.dt.float32
    nc = bacc.Bacc("TRN2", target_bir_lowering=False)
    tbl_in = nc.dram_tensor("tbl", [G, B], f32, kind="ExternalInput")  # x.T
    idx_in = nc.dram_tensor(
        "idx", [128, NCALL * NIDX // 16], mybir.dt.int16, kind="ExternalInput"
    )
    c_out = nc.dram_tensor("c", [128, 128], f32, kind="ExternalOutput")

    with tile.TileContext(nc) as tc:
        with (
            tc.tile_pool(name="singles", bufs=1) as singles,
            tc.tile_pool(name="gath", bufs=3) as gath,
            tc.tile_pool(name="work", bufs=2) as work,
            tc.tile_pool(name="small", bufs=2) as small,
        ):
            idxs = singles.tile([128, NCALL * NIDX // 16], mybir.dt.int16)
            nc.sync.dma_start(out=idxs, in_=idx_in[:, :])
            vv = singles.tile([128, GRP_PER_PART, B], f32)  # v = 1000*softand
            for ch in range(NCHUNK):
                gt = gath.tile([128, CALLS_PER_CHUNK * 8, B], f32)
                for ci in range(CALLS_PER_CHUNK):
                    c = ch * CALLS_PER_CHUNK + ci
                    nc.gpsimd.dma_gather(
                        gt[:, ci * 8 : (ci + 1) * 8, :],
                        tbl_in[:, :],
                        idxs[:, c * (NIDX // 16) : (c + 1) * (NIDX // 16)],
                        num_idxs=NIDX,
                        num_idxs_reg=NIDX,
                        elem_size=B,
                    )
                # gt slots = (grp K, l 8), b innermost: strides grp 8B, l B, b 1
                K = CALLS_PER_CHUNK
                m = work.tile([128, K, B], f32)
                nc.vector.tensor_reduce(
                    out=m,
                    in_=_v(gt, [[8 * B, K], [1, B], [B, 8]]),  # [grp, b, l]
                    axis=mybir.AxisListType.X,
                    op=mybir.AluOpType.min,
                )
                d = work.tile([128, K, 8, B], f32)
                nc.vector.tensor_tensor(
                    out=d,
                    in0=_v(m, [[B, K], [0, 8], [1, B]]),  # m bcast over l
                    in1=_v(gt, [[8 * B, K], [B, 8], [1, B]]),  # [grp, l, b]
                    op=mybir.AluOpType.subtract,
                )  # m - g  (<= 0)
                e = work.tile([128, K, 8, B], f32)
                nc.scalar.activation(
                    out=e, in_=d, func=mybir.ActivationFunctionType.Exp, scale=1000.0
                )
                s_ = work.tile([128, K, B], f32)
                nc.vector.tensor_reduce(
                    out=s_,
                    in_=_v(e, [[8 * B, K], [1, B], [B, 8]]),  # [grp, b, l]
                    axis=mybir.AxisListType.X,
                    op=mybir.AluOpType.add,
                )
                ls = small.tile([128, K, B], f32)
                nc.scalar.activation(
                    out=ls, in_=s_, func=mybir.ActivationFunctionType.Ln
                )
                mt = small.tile([128, K, B], f32)
                nc.scalar.activation(
                    out=mt, in_=m, func=mybir.ActivationFunctionType.Copy, scale=1000.0
                )
                nc.vector.tensor_tensor(
                    out=vv[:, ch * K : (ch + 1) * K, :],
                    in0=mt,
                    in1=ls,
                    op=mybir.AluOpType.subtract,
                )  # v = 1000*m - ln S
            # stage 2 over s=32; vv grp = gl*32 + s -> gl stride 32B, s stride B
            vm = small.tile([128, 2, B], f32)
            nc.vector.tensor_reduce(
                out=vm,
                in_=_v(vv, [[32 * B, 2], [1, B], [B, 32]]),  # [gl, b, s]
                axis=mybir.AxisListType.X,
                op=mybir.AluOpType.max,
            )
            d2 = singles.tile([128, 2, 32, B], f32)
            nc.vector.tensor_tensor(
                out=d2,
                in0=_v(vv, [[32 * B, 2], [B, 32], [1, B]]),  # [gl, s, b]
                in1=_v(vm, [[B, 2], [0, 32], [1, B]]),  # M bcast over s
                op=mybir.AluOpType.subtract,
            )  # v - M (<= 0)
            e2 = singles.tile([128, 2, 32, B], f32)
            nc.scalar.activation(out=e2, in_=d2, func=mybir.ActivationFunctionType.Exp)
            s2 = small.tile([128, 2, B], f32)
            nc.vector.tensor_reduce(
                out=s2,
                in_=_v(e2, [[32 * B, 2], [1, B], [B, 32]]),  # [gl, b, s]
                axis=mybir.AxisListType.X,
                op=mybir.AluOpType.add,
            )
            l2 = small.tile([128, 2, B], f32)
            nc.scalar.activation(out=l2, in_=s2, func=mybir.ActivationFunctionType.Ln)
            c1000 = small.tile([128, 2, B], f32)
            nc.vector.tensor_tensor(out=c1000, in0=vm, in1=l2, op=mybir.AluOpType.add)
            cf = small.tile([128, 2, B], f32)
            nc.scalar.activation(
                out=cf, in_=c1000, func=mybir.ActivationFunctionType.Copy, scale=0.001
            )
            nc.sync.dma_start(out=c_out[:, :], in_=_v(cf, [[1, 2 * B]]))
    nc.finalize()
    return nc


def _prep_inputs(x: np.ndarray, I_i: np.ndarray):
    """Host-side layout: x transposed; per-core wrapped idx tensors."""
    tbl = np.ascontiguousarray(x.astype(np.float32, copy=False).T)  # [G, B]
    idx_maps = []
    I = np.asarray(I_i)
    for k in range(NCORES):
        Ik = I[k * GSH : (k + 1) * GSH]  # [256, 32, 8] values in [0, G)
        # call c gathers l=0..7 of group c for every partition p.
        # group c = gl*32 + s ; partition p holds g' = 2p + gl
        # list position j = i*128 + p  (i = l)
        Ikr = Ik.reshape(128, 2, S, L)  # [p, gl, s, l]
        lc = np.transpose(Ikr, (1, 2, 3, 0)).reshape(2 * S, L, 128)  # [c, i, p]
        flat = lc.reshape(NCALL, NIDX)  # j = i*128+p
        # wrapped: partition q slot t of call c holds flat[c, t*16 + q%16]
        w = flat.reshape(NCALL, NIDX // 16, 16)  # [c, t, q%16]
        w = np.transpose(w, (2, 0, 1)).reshape(16, NCALL * (NIDX // 16))
        idx = np.tile(w, (8, 1)).astype(np.int16)  # replicate to 128 partitions
        idx_maps.append(idx)
    return tbl, idx_maps


def kernel(x: np.ndarray, I_i: np.ndarray) -> np.ndarray:
    global _nc_cache, last_result
    if _nc_cache is None:
        _nc_cache = _build_nc()
    nc = _nc_cache
    tbl, idx_maps = _prep_inputs(x, I_i)
    in_maps = [{"tbl": tbl, "idx": idx_maps[k]} for k in range(NCORES)]
    res = run_bass_kernel_spmd(nc, in_maps, core_ids=list(range(NCORES)))
    last_result = res
    C = np.empty((B, G), dtype=np.float32)
    for k in range(NCORES):
        o = res.results[k]["c"].reshape(128, 2, B)  # [p, gl, b]
        C[:, k * GSH : (k + 1) * GSH] = np.transpose(o, (2, 0, 1)).reshape(B, GSH)
    return C


# revision 9
# speedup vs baseline: 1.0218x; 1.0218x over previous
"""Trainium2 kernel for nn_ClauseFunction (segment_reduce):
C[b,g] = softor_s(softand_l(x[b, I_i[g,s,l]])), gamma=1e-3.

Strategy: shard over G (each of 8 cores handles 256 g-columns; x replicated).
Per core: gather 256*32*8 = 65536 rows of xT (one row = x[:,j] for all 64 b,
256 bytes f32) from DRAM via gpsimd.dma_gather (64 calls x 1024 idxs), then
logsumexp reductions on DVE/ACT:
  stage1 (over l=8):  m=min_l g; S=sum_l exp((m-g)*1000); v=1000*m - ln S
  stage2 (over s=32): M=max_s v; C=1e-3*(M + ln sum_s exp(v-M))
Layout: gathered tile [128 part, slots, 64 b]; partition p holds g' in
{2p, 2p+1}; slot group c = gl*32+s (gl=g' parity, s); call c gathers l=0..7
for group c of every partition.
"""

import numpy as np

import concourse.bacc as bacc
import concourse.bass as bass
import concourse.tile as tile
from concourse import mybir
from concourse.bass_utils import run_bass_kernel_spmd

B, G, S, L = 64, 2048, 32, 8
NCORES = 8
GSH = G // NCORES  # 256 g' per core
NIDX = 1024  # indices per dma_gather call (ucode scratch-safe)
NCALL = (GSH * S * L) // NIDX  # 64 calls
# chunk sizes (calls per chunk); tapered so each half's final compute tail is
# short, and each half (32 calls) ends on a gl boundary so stage 2 for that
# half overlaps the other half's gathers.
CHUNK_SIZES = [4] * 7 + [2, 2] + [4] * 7 + [2, 2]
GRP_PER_PART = GSH // 128 * S  # 64 groups (gl, s) per partition

_nc_cache = None
last_result = None


def _v(t, dims, off=0):
    """View of tile t with explicit free-dim [stride, count] pairs (elements).

    Keeps the tile's own partition entry (stride = per-partition size)."""
    return bass.AP(tensor=t.tensor, offset=t.offset + off, ap=[list(t.ap[0])] + dims)


def _stage2(nc, tc, small, vv, c_out, gl):
    """softor over s for half gl of vv; writes c_out columns [gl*64,(gl+1)*64)."""
    f32 = mybir.dt.float32
    off = gl * 32 * B
    vm = small.tile([128, B], f32, tag="vm")
    nc.vector.tensor_reduce(
        out=vm,
        in_=_v(vv, [[1, B], [B, 32]], off),  # [b, s]
        axis=mybir.AxisListType.X,
        op=mybir.AluOpType.max,
    )
    d2 = small.tile([128, 32, B], f32, tag="d2")
    nc.vector.tensor_tensor(
        out=d2,
        in0=_v(vv, [[B, 32], [1, B]], off),  # [s, b]
        in1=_v(vm, [[0, 32], [1, B]]),  # M bcast over s
        op=mybir.AluOpType.subtract,
    )  # v - M (<= 0)
    e2 = small.tile([128, 32, B], f32, tag="e2")
    nc.scalar.activation(out=e2, in_=d2, func=mybir.ActivationFunctionType.Exp)
    s2 = small.tile([128, B], f32, tag="s2")
    nc.vector.tensor_reduce(
        out=s2,
        in_=_v(e2, [[1, B], [B, 32]]),  # [b, s]
        axis=mybir.AxisListType.X,
        op=mybir.AluOpType.add,
    )
    l2 = small.tile([128, B], f32, tag="l2")
    nc.scalar.activation(out=l2, in_=s2, func=mybir.ActivationFunctionType.Ln)
    c1000 = small.tile([128, B], f32, tag="c1000")
    nc.vector.tensor_tensor(out=c1000, in0=vm, in1=l2, op=mybir.AluOpType.add)
    cf = small.tile([128, B], f32, tag="cf")
    nc.scalar.activation(
        out=cf, in_=c1000, func=mybir.ActivationFunctionType.Copy, scale=0.001
    )
    nc.sync.dma_start(out=c_out[:, gl * B : (gl + 1) * B], in_=cf)


def _build_nc():
    f32 = mybir.dt.float32
    nc = bacc.Bacc("TRN2", target_bir_lowering=False)
    tbl_in = nc.dram_tensor("tbl", [G, B], f32, kind="ExternalInput")  # x.T
    idx_in = nc.dram_tensor(
        "idx", [128, NCALL * NIDX // 16], mybir.dt.int16, kind="ExternalInput"
    )
    c_out = nc.dram_tensor("c", [128, 128], f32, kind="ExternalOutput")

    with tile.TileContext(nc) as tc:
        with (
            tc.tile_pool(name="singles", bufs=1) as singles,
            tc.tile_pool(name="gath", bufs=3) as gath,
            tc.tile_pool(name="work", bufs=2) as work,
            tc.tile_pool(name="small", bufs=2) as small,
        ):
            idxs = singles.tile([128, NCALL * NIDX // 16], mybir.dt.int16)
            # split the idx load so the first gather can start early
            first_cols = CHUNK_SIZES[0] * (NIDX // 16)
            nc.sync.dma_start(out=idxs[:, :first_cols], in_=idx_in[:, :first_cols])
            nc.sync.dma_start(out=idxs[:, first_cols:], in_=idx_in[:, first_cols:])
            vv = singles.tile([128, GRP_PER_PART, B], f32)  # v = 1000*softand
            call_base = 0
            for ch, K in enumerate(CHUNK_SIZES):
                gt = gath.tile([128, max(CHUNK_SIZES) * 8, B], f32, tag="gt")
                for ci in range(K):
                    c = call_base + ci
                    nc.gpsimd.dma_gather(
                        gt[:, ci * 8 : (ci + 1) * 8, :],
                        tbl_in[:, :],
                        idxs[:, c * (NIDX // 16) : (c + 1) * (NIDX // 16)],
                        num_idxs=NIDX,
                        num_idxs_reg=NIDX,
                        elem_size=B,
                    )
                # gt slots = (grp K, l 8), b innermost: strides grp 8B, l B, b 1
                m = work.tile([128, max(CHUNK_SIZES), B], f32, tag="m")
                nc.vector.tensor_reduce(
                    out=m[:, :K, :],
                    in_=_v(gt, [[8 * B, K], [1, B], [B, 8]]),  # [grp, b, l]
                    axis=mybir.AxisListType.X,
                    op=mybir.AluOpType.min,
                )
                d = work.tile([128, max(CHUNK_SIZES), 8, B], f32, tag="d")
                nc.vector.tensor_tensor(
                    out=d[:, :K, :, :],
                    in0=_v(m, [[B, K], [0, 8], [1, B]]),  # m bcast over l
                    in1=_v(gt, [[8 * B, K], [B, 8], [1, B]]),  # [grp, l, b]
                    op=mybir.AluOpType.subtract,
                )  # m - g  (<= 0)
                e = work.tile([128, max(CHUNK_SIZES), 8, B], f32, tag="e")
                nc.scalar.activation(
                    out=e[:, :K, :, :],
                    in_=d[:, :K, :, :],
                    func=mybir.ActivationFunctionType.Exp,
                    scale=1000.0,
                )
                s_ = work.tile([128, max(CHUNK_SIZES), B], f32, tag="s_")
                nc.vector.tensor_reduce(
                    out=s_[:, :K, :],
                    in_=_v(e, [[8 * B, K], [1, B], [B, 8]]),  # [grp, b, l]
                    axis=mybir.AxisListType.X,
                    op=mybir.AluOpType.add,
                )
                ls = small.tile([128, max(CHUNK_SIZES), B], f32, tag="ls")
                nc.scalar.activation(
                    out=ls[:, :K, :],
                    in_=s_[:, :K, :],
                    func=mybir.ActivationFunctionType.Ln,
                )
                mt = small.tile([128, max(CHUNK_SIZES), B], f32, tag="mt")
                nc.scalar.activation(
                    out=mt[:, :K, :],
                    in_=m[:, :K, :],
                    func=mybir.ActivationFunctionType.Copy,
                    scale=1000.0,
                )
                nc.vector.tensor_tensor(
                    out=vv[:, call_base : call_base + K, :],
                    in0=mt[:, :K, :],
                    in1=ls[:, :K, :],
                    op=mybir.AluOpType.subtract,
                )  # v = 1000*m - ln S
                call_base += K
                if call_base % 32 == 0:
                    _stage2(nc, tc, small, vv, c_out, call_base // 32 - 1)
    nc.finalize()
    return nc


def _prep_inputs(x: np.ndarray, I_i: np.ndarray):
    """Host-side layout: x transposed; per-core wrapped idx tensors."""
    tbl = np.ascontiguousarray(x.astype(np.float32, copy=False).T)  # [G, B]
    idx_maps = []
    I = np.asarray(I_i)
    for k in range(NCORES):
        Ik = I[k * GSH : (k + 1) * GSH]  # [256, 32, 8] values in [0, G)
        # call c gathers l=0..7 of group c for every partition p.
        # group c = gl*32 + s ; partition p holds g' = 2p + gl
        # list position j = i*128 + p  (i = l)
        Ikr = Ik.reshape(128, 2, S, L)  # [p, gl, s, l]
        lc = np.transpose(Ikr, (1, 2, 3, 0)).reshape(2 * S, L, 128)  # [c, i, p]
        flat = lc.reshape(NCALL, NIDX)  # j = i*128+p
        # wrapped: partition q slot t of call c holds flat[c, t*16 + q%16]
        w = flat.reshape(NCALL, NIDX // 16, 16)  # [c, t, q%16]
        w = np.transpose(w, (2, 0, 1)).reshape(16, NCALL * (NIDX // 16))
        idx = np.tile(w, (8, 1)).astype(np.int16)  # replicate to 128 partitions
        idx_maps.append(idx)
    return tbl, idx_maps


def kernel(x: np.ndarray, I_i: np.ndarray) -> np.ndarray:
    global _nc_cache, last_result
    if _nc_cache is None:
        _nc_cache = _build_nc()
    nc = _nc_cache
    tbl, idx_maps = _prep_inputs(x, I_i)
    in_maps = [{"tbl": tbl, "idx": idx_maps[k]} for k in range(NCORES)]
    res = run_bass_kernel_spmd(nc, in_maps, core_ids=list(range(NCORES)))
    last_result = res
    C = np.empty((B, G), dtype=np.float32)
    for k in range(NCORES):
        o = res.results[k]["c"].reshape(128, 2, B)  # [p, gl, b]
        C[:, k * GSH : (k + 1) * GSH] = np.transpose(o, (2, 0, 1)).reshape(B, GSH)
    return C


# revision 10
# speedup vs baseline: 1.0347x; 1.0126x over previous
"""Trainium2 kernel for nn_ClauseFunction (segment_reduce):
C[b,g] = softor_s(softand_l(x[b, I_i[g,s,l]])), gamma=1e-3.

Strategy: shard over G (each of 8 cores handles 256 g-columns; x replicated).
Per core: gather 256*32*8 = 65536 rows of xT (one row = x[:,j] for all 64 b,
256 bytes f32) from DRAM via gpsimd.dma_gather (64 calls x 1024 idxs), then
logsumexp reductions on DVE/ACT:
  stage1 (over l=8):  m=min_l g; S=sum_l exp((m-g)*1000); v=1000*m - ln S
  stage2 (over s=32): M=max_s v; C=1e-3*(M + ln sum_s exp(v-M))
Layout: gathered tile [128 part, slots, 64 b]; partition p holds g' in
{2p, 2p+1}; slot group c = gl*32+s (gl=g' parity, s); call c gathers l=0..7
for group c of every partition.
"""

import numpy as np

import concourse.bacc as bacc
import concourse.bass as bass
import concourse.tile as tile
from concourse import mybir
from concourse.bass_utils import run_bass_kernel_spmd

B, G, S, L = 64, 2048, 32, 8
NCORES = 8
GSH = G // NCORES  # 256 g' per core
NIDX = 1024  # indices per dma_gather call (ucode scratch-safe)
NCALL = (GSH * S * L) // NIDX  # 64 calls
# chunk sizes (calls per chunk); tapered so each half's final compute tail is
# short, and each half (32 calls) ends on a gl boundary so stage 2 for that
# half overlaps the other half's gathers.
CHUNK_SIZES = [4] * 7 + [2, 1, 1] + [4] * 7 + [2, 1, 1]
GRP_PER_PART = GSH // 128 * S  # 64 groups (gl, s) per partition

_nc_cache = None
last_result = None


def _v(t, dims, off=0):
    """View of tile t with explicit free-dim [stride, count] pairs (elements).

    Keeps the tile's own partition entry (stride = per-partition size)."""
    return bass.AP(tensor=t.tensor, offset=t.offset + off, ap=[list(t.ap[0])] + dims)


def _stage2(nc, tc, small, vv, c_out, gl):
    """softor over s for half gl of vv; writes c_out columns [gl*64,(gl+1)*64)."""
    f32 = mybir.dt.float32
    off = gl * 32 * B
    vm = small.tile([128, B], f32, tag="vm")
    nc.vector.tensor_reduce(
        out=vm,
        in_=_v(vv, [[1, B], [B, 32]], off),  # [b, s]
        axis=mybir.AxisListType.X,
        op=mybir.AluOpType.max,
    )
    d2 = small.tile([128, 32, B], f32, tag="d2")
    nc.vector.tensor_tensor(
        out=d2,
        in0=_v(vv, [[B, 32], [1, B]], off),  # [s, b]
        in1=_v(vm, [[0, 32], [1, B]]),  # M bcast over s
        op=mybir.AluOpType.subtract,
    )  # v - M (<= 0)
    e2 = small.tile([128, 32, B], f32, tag="e2")
    nc.scalar.activation(out=e2, in_=d2, func=mybir.ActivationFunctionType.Exp)
    s2 = small.tile([128, B], f32, tag="s2")
    nc.vector.tensor_reduce(
        out=s2,
        in_=_v(e2, [[1, B], [B, 32]]),  # [b, s]
        axis=mybir.AxisListType.X,
        op=mybir.AluOpType.add,
    )
    l2 = small.tile([128, B], f32, tag="l2")
    nc.scalar.activation(out=l2, in_=s2, func=mybir.ActivationFunctionType.Ln)
    c1000 = small.tile([128, B], f32, tag="c1000")
    nc.vector.tensor_tensor(out=c1000, in0=vm, in1=l2, op=mybir.AluOpType.add)
    cf = small.tile([128, B], f32, tag="cf")
    nc.scalar.activation(
        out=cf, in_=c1000, func=mybir.ActivationFunctionType.Copy, scale=0.001
    )
    nc.sync.dma_start(out=c_out[:, gl * B : (gl + 1) * B], in_=cf)


def _build_nc():
    f32 = mybir.dt.float32
    nc = bacc.Bacc("TRN2", target_bir_lowering=False)
    tbl_in = nc.dram_tensor("tbl", [G, B], f32, kind="ExternalInput")  # x.T
    idx_in = nc.dram_tensor(
        "idx", [128, NCALL * NIDX // 16], mybir.dt.int16, kind="ExternalInput"
    )
    c_out = nc.dram_tensor("c", [128, 128], f32, kind="ExternalOutput")

    with tile.TileContext(nc) as tc:
        with (
            tc.tile_pool(name="singles", bufs=1) as singles,
            tc.tile_pool(name="gath", bufs=3) as gath,
            tc.tile_pool(name="work", bufs=2) as work,
            tc.tile_pool(name="small", bufs=2) as small,
        ):
            idxs = singles.tile([128, NCALL * NIDX // 16], mybir.dt.int16)
            # split the idx load so the first gather can start early
            first_cols = CHUNK_SIZES[0] * (NIDX // 16)
            nc.sync.dma_start(out=idxs[:, :first_cols], in_=idx_in[:, :first_cols])
            nc.sync.dma_start(out=idxs[:, first_cols:], in_=idx_in[:, first_cols:])
            vv = singles.tile([128, GRP_PER_PART, B], f32)  # v = 1000*softand
            call_base = 0
            for ch, K in enumerate(CHUNK_SIZES):
                gt = gath.tile([128, max(CHUNK_SIZES) * 8, B], f32, tag="gt")
                for ci in range(K):
                    c = call_base + ci
                    nc.gpsimd.dma_gather(
                        gt[:, ci * 8 : (ci + 1) * 8, :],
                        tbl_in[:, :],
                        idxs[:, c * (NIDX // 16) : (c + 1) * (NIDX // 16)],
                        num_idxs=NIDX,
                        num_idxs_reg=NIDX,
                        elem_size=B,
                    )
                # gt slots = (grp K, l 8), b innermost: strides grp 8B, l B, b 1
                m = work.tile([128, max(CHUNK_SIZES), B], f32, tag="m")
                nc.vector.tensor_reduce(
                    out=m[:, :K, :],
                    in_=_v(gt, [[8 * B, K], [1, B], [B, 8]]),  # [grp, b, l]
                    axis=mybir.AxisListType.X,
                    op=mybir.AluOpType.min,
                )
                d = work.tile([128, max(CHUNK_SIZES), 8, B], f32, tag="d")
                nc.vector.tensor_tensor(
                    out=d[:, :K, :, :],
                    in0=_v(m, [[B, K], [0, 8], [1, B]]),  # m bcast over l
                    in1=_v(gt, [[8 * B, K], [B, 8], [1, B]]),  # [grp, l, b]
                    op=mybir.AluOpType.subtract,
                )  # m - g  (<= 0)
                e = work.tile([128, max(CHUNK_SIZES), 8, B], f32, tag="e")
                nc.scalar.activation(
                    out=e[:, :K, :, :],
                    in_=d[:, :K, :, :],
                    func=mybir.ActivationFunctionType.Exp,
                    scale=1000.0,
                )
                s_ = work.tile([128, max(CHUNK_SIZES), B], f32, tag="s_")
                nc.vector.tensor_reduce(
                    out=s_[:, :K, :],
                    in_=_v(e, [[8 * B, K], [1, B], [B, 8]]),  # [grp, b, l]
                    axis=mybir.AxisListType.X,
                    op=mybir.AluOpType.add,
                )
                ls = small.tile([128, max(CHUNK_SIZES), B], f32, tag="ls")
                nc.scalar.activation(
                    out=ls[:, :K, :],
                    in_=s_[:, :K, :],
                    func=mybir.ActivationFunctionType.Ln,
                )
                mt = small.tile([128, max(CHUNK_SIZES), B], f32, tag="mt")
                nc.scalar.activation(
                    out=mt[:, :K, :],
                    in_=m[:, :K, :],
                    func=mybir.ActivationFunctionType.Copy,
                    scale=1000.0,
                )
                nc.vector.tensor_tensor(
                    out=vv[:, call_base : call_base + K, :],
                    in0=mt[:, :K, :],
                    in1=ls[:, :K, :],
                    op=mybir.AluOpType.subtract,
                )  # v = 1000*m - ln S
                call_base += K
                if call_base % 32 == 0:
                    _stage2(nc, tc, small, vv, c_out, call_base // 32 - 1)
    nc.finalize()
    return nc


def _prep_inputs(x: np.ndarray, I_i: np.ndarray):
    """Host-side layout: x transposed; per-core wrapped idx tensors."""
    tbl = np.ascontiguousarray(x.astype(np.float32, copy=False).T)  # [G, B]
    idx_maps = []
    I = np.asarray(I_i)
    for k in range(NCORES):
        Ik = I[k * GSH : (k + 1) * GSH]  # [256, 32, 8] values in [0, G)
        # call c gathers l=0..7 of group c for every partition p.
        # group c = gl*32 + s ; partition p holds g' = 2p + gl
        # list position j = i*128 + p  (i = l)
        Ikr = Ik.reshape(128, 2, S, L)  # [p, gl, s, l]
        lc = np.transpose(Ikr, (1, 2, 3, 0)).reshape(2 * S, L, 128)  # [c, i, p]
        flat = lc.reshape(NCALL, NIDX)  # j = i*128+p
        # wrapped: partition q slot t of call c holds flat[c, t*16 + q%16]
        w = flat.reshape(NCALL, NIDX // 16, 16)  # [c, t, q%16]
        w = np.transpose(w, (2, 0, 1)).reshape(16, NCALL * (NIDX // 16))
        idx = np.tile(w, (8, 1)).astype(np.int16)  # replicate to 128 partitions
        idx_maps.append(idx)
    return tbl, idx_maps


def kernel(x: np.ndarray, I_i: np.ndarray) -> np.ndarray:
    global _nc_cache, last_result
    if _nc_cache is None:
        _nc_cache = _build_nc()
    nc = _nc_cache
    tbl, idx_maps = _prep_inputs(x, I_i)
    in_maps = [{"tbl": tbl, "idx": idx_maps[k]} for k in range(NCORES)]
    res = run_bass_kernel_spmd(nc, in_maps, core_ids=list(range(NCORES)))
    last_result = res
    C = np.empty((B, G), dtype=np.float32)
    for k in range(NCORES):
        o = res.results[k]["c"].reshape(128, 2, B)  # [p, gl, b]
        C[:, k * GSH : (k + 1) * GSH] = np.transpose(o, (2, 0, 1)).reshape(B, GSH)
    return C
